# revision 4
# baseline (speedup 1.0000x reference)
"""Trainium2 Bass kernel for nn_MinimalSSM: selective-scan SSM block.

Reference computation (per batch b):
    proj  = x @ W_xproj + b_xproj                # [L, d+2n]
    delta = softplus(proj[:, :d])                # [L, d]
    Bm, Cm = proj[:, d:d+n], proj[:, d+n:]       # [L, n]
    A     = -exp(A_log)                          # [d, n]
    h_t   = exp(delta_t*A) * h_{t-1} + delta_t*Bm_t*x_t   (elementwise [d, n])
    y_t   = sum_n(h_t * Cm_t) + Dp * x_t
    out   = y @ W_out + b_out

Sharding (8 cores): batch (4) x d_model-half (2).  Each core computes the
full recurrence for its 512 channels of its batch, and a partial
out-projection (contraction over its d-half).  A second tiny kernel adds the
two partials per batch (sharded batch x out-column-half).

The time recurrence runs on VectorE's native tensor_tensor_scan
(state = a*state + b along the free dim, fp32 internal state).
exp(delta*A) is computed on ScalarE as activation(Exp, scale=A[:,n]) --
per-partition vector scale -- so the DVE only does the multiplies,
scan, and the n-reduction tree.

Layout inside a core: partition dim = 128-channel block (4 blocks),
free dims = (n=16, t=Tc) per time-chunk, t innermost/contiguous.
"""

import numpy as np
import ml_dtypes

import concourse.bacc as bacc
import concourse.bass as bass
import concourse.tile as tile
from concourse import mybir
from concourse.bass_utils import run_bass_kernel_spmd
from contextlib import ExitStack

F32 = mybir.dt.float32
BF16 = mybir.dt.bfloat16
FP16 = mybir.dt.float16
AF = mybir.ActivationFunctionType
OP = mybir.AluOpType

B, T, D, N = 4, 2048, 1024, 16
DL = D // 2          # channels per core
NJB = DL // 128      # 4 local channel blocks
NKB = D // 128       # 8 contraction blocks for proj
NEB = D // 128       # 8 output-column blocks
PC = 544             # proj columns per core: 512 delta + 16 B + 16 C
TC = 256             # time chunk
NCH = T // TC

_cache = {}


def _pin_act_tables():
    """Restrict bacc's activation-table choices to the one set containing
    every function we use (Exp, Ln, Identity, MemsetZero) so the compiler
    never inserts mid-kernel ACT_TABLE_LOAD switches."""
    import concourse.bacc as _bacc_mod
    from concourse.hw_specs import get_activation_tables as _orig

    def _only_nl_exp(arch):
        tabs = _orig(arch)
        # keep every entry (act_func_set_id is positional) but empty out the
        # alternatives so the chooser can only pick the one full set
        return {k: (v if k == "natural_log_exp_and_others" else set())
                for k, v in tabs.items()}

    _bacc_mod.get_activation_tables = _only_nl_exp


_pin_act_tables()


def _build_stage1(t_len=T, tc=TC):
    nch = t_len // tc
    nc = bacc.Bacc("TRN2", target_bir_lowering=False, debug=False, num_devices=8)
    xt = nc.dram_tensor("xt", [D, t_len], BF16, kind="ExternalInput")
    wx = nc.dram_tensor("wx", [D, PC], BF16, kind="ExternalInput")
    bx = nc.dram_tensor("bx", [128, 5], F32, kind="ExternalInput")
    alog = nc.dram_tensor("alog", [128, NJB * N], F32, kind="ExternalInput")
    dp = nc.dram_tensor("dp", [128, NJB], F32, kind="ExternalInput")
    wo = nc.dram_tensor("wo", [DL, D], BF16, kind="ExternalInput")
    bo = nc.dram_tensor("bo", [128, NEB], F32, kind="ExternalInput")
    ident = nc.dram_tensor("ident", [128, 128], BF16, kind="ExternalInput")
    part = nc.dram_tensor("part", [D, t_len], FP16, kind="ExternalOutput")
    bc_dram = nc.dram_tensor("bc_scratch", [t_len // tc, 32, tc], BF16)

    with tile.TileContext(nc) as tc_ctx, ExitStack() as ctx:
        const = ctx.enter_context(tc_ctx.tile_pool(name="const", bufs=1))
        psum = ctx.enter_context(
            tc_ctx.tile_pool(name="psum", bufs=8, space="PSUM"))
        dpool = ctx.enter_context(tc_ctx.tile_pool(name="delta", bufs=6))
        dxpool = ctx.enter_context(tc_ctx.tile_pool(name="dx", bufs=6))
        bcpool = ctx.enter_context(tc_ctx.tile_pool(name="bc", bufs=2))
        reppool = ctx.enter_context(tc_ctx.tile_pool(name="rep", bufs=2))
        apool = ctx.enter_context(tc_ctx.tile_pool(name="apool", bufs=3))
        workpool = ctx.enter_context(tc_ctx.tile_pool(name="work", bufs=4))
        hpool = ctx.enter_context(tc_ctx.tile_pool(name="hpool", bufs=4))
        cpool = ctx.enter_context(tc_ctx.tile_pool(name="carry", bufs=8))
        ypool = ctx.enter_context(tc_ctx.tile_pool(name="y", bufs=4))
        ybfpool = ctx.enter_context(tc_ctx.tile_pool(name="ybf", bufs=8))
        popool = ctx.enter_context(tc_ctx.tile_pool(name="po", bufs=3))

        xt_sb = []
        for kb in range(NKB):
            tt = const.tile([128, t_len], BF16, tag=f"xt{kb}")
            nc.sync.dma_start(tt[:, 0:tc], xt[kb * 128:(kb + 1) * 128, 0:tc])
            xt_sb.append(tt)
        for kb in range(NKB):
            nc.sync.dma_start(xt_sb[kb][:, tc:],
                              xt[kb * 128:(kb + 1) * 128, tc:])
        wx_sb = []
        for kb in range(NKB):
            tt = const.tile([128, PC], BF16, tag=f"wx{kb}")
            nc.sync.dma_start(tt[:], wx[kb * 128:(kb + 1) * 128, :])
            wx_sb.append(tt)
        wo_sb = []
        for kb in range(NJB):
            tt = const.tile([128, D], BF16, tag=f"wo{kb}")
            nc.sync.dma_start(tt[:], wo[kb * 128:(kb + 1) * 128, :])
            wo_sb.append(tt)
        bx_sb = const.tile([128, 5], F32, tag="bx")
        nc.sync.dma_start(bx_sb[:], bx[:])
        bo_sb = const.tile([128, NEB], F32, tag="bo")
        nc.sync.dma_start(bo_sb[:], bo[:])
        dp_sb = const.tile([128, NJB], F32, tag="dp")
        nc.sync.dma_start(dp_sb[:], dp[:])
        alog_sb = const.tile([128, NJB * N], F32, tag="alog")
        nc.sync.dma_start(alog_sb[:], alog[:])
        aexp_sb = const.tile([128, NJB * N], F32, tag="aexp")
        nc.scalar.activation(aexp_sb[:], alog_sb[:], AF.Exp)
        aneg_sb = const.tile([128, NJB * N], F32, tag="aneg")
        nc.vector.tensor_scalar_mul(aneg_sb[:], aexp_sb[:], -1.0)
        id_sb = const.tile([128, 128], BF16, tag="ident")
        nc.sync.dma_start(id_sb[:], ident[:])
        # (0, 1) bf16 pattern for the chained-scan dummy columns
        const01 = const.tile([128, N, 2], BF16, tag="const01")
        nc.vector.memset(const01[:, :, 0:1], 0.0)
        nc.vector.memset(const01[:, :, 1:2], 1.0)

        carry_tiles = [None] * NJB
        pending_out = None
        for ci in range(nch):
            t0 = ci * tc
            delta_tiles = []
            bct = None
            for m in (4, 0, 1, 2, 3):   # B/C first: unblocks the broadcasts
                mm = 128 if m < 4 else 32
                ps = psum.tile([mm, tc], F32, tag="ps")
                for kb in range(NKB):
                    nc.tensor.matmul(
                        ps[:],
                        wx_sb[kb][:, m * 128:m * 128 + mm],
                        xt_sb[kb][:, t0:t0 + tc],
                        start=(kb == 0), stop=(kb == NKB - 1))
                if m < 4:
                    # softplus(v) = ln(1 + exp(v)); Exp+Ln share one ACT table
                    et_ = dpool.tile([128, tc], F32, tag="etmp")
                    nc.scalar.activation(et_[:], ps[:], AF.Exp,
                                         bias=bx_sb[:, m:m + 1])
                    dt_ = dpool.tile([128, tc], BF16, tag="delta")
                    nc.scalar.activation(dt_[:], et_[:], AF.Ln, bias=1.0)
                    delta_tiles.append(dt_)
                else:
                    bct = bcpool.tile([32, tc], BF16, tag="bc")
                    nc.scalar.activation(bct[:], ps[:], AF.Identity,
                                         bias=bx_sb[:32, 4:5])
            nc.sync.dma_start(bc_dram[ci], bct[:])
            brep = reppool.tile([128, N, tc], BF16, tag="brep")
            crep = reppool.tile([128, N, tc], BF16, tag="crep")
            nc.sync.dma_start(brep[:],
                              bc_dram[ci, 0:N, :].partition_broadcast(128))
            nc.sync.dma_start(crep[:],
                              bc_dram[ci, N:2 * N, :].partition_broadcast(128))

            y_tiles = []
            carry_jobs = []
            for jb in range(NJB):
                dt_ = delta_tiles[jb]
                dxt = dxpool.tile([128, tc], BF16, tag="dx")
                nc.vector.tensor_mul(dxt[:], dt_[:], xt_sb[jb][:, t0:t0 + tc])
                # a/b tiles carry 2 leading dummy columns per n-segment:
                # a = (0, 1), b = (carry, 0).  One flat scan then chains all
                # 16 n-segments: the a=0 column resets the running state to
                # the injected carry, the (1, 0) column passes it through.
                tcp = tc + 2
                at = apool.tile([128, N, tcp], BF16, tag="a")
                nc.vector.tensor_copy(at[:, :, 0:2], const01[:])
                for n in range(N):
                    nc.scalar.activation(
                        at[:, n, 2:], dt_[:], AF.Exp,
                        scale=aneg_sb[:, jb * N + n:jb * N + n + 1])
                bt = workpool.tile([128, N, tcp], BF16, tag="work")
                if ci == 0:
                    nc.vector.memset(bt[:, :, 0:2], 0.0)
                else:
                    nc.vector.tensor_copy(bt[:, :, 0:2], carry_tiles[jb][:])
                dx_b = dxt[:].unsqueeze(1).broadcast_to([128, N, tc])
                nc.vector.tensor_mul(bt[:, :, 2:], dx_b, brep[:])
                ht = hpool.tile([128, N, tcp], BF16, tag="h")
                nc.vector.tensor_tensor_scan(
                    ht[:].rearrange("p n t -> p (n t)"),
                    at[:].rearrange("p n t -> p (n t)"),
                    bt[:].rearrange("p n t -> p (n t)"),
                    0.0, op0=OP.mult, op1=OP.add)
                if ci < nch - 1:
                    carry_jobs.append((jb, ht))
                # extract the PREVIOUS jb's carry on ACT: by now its scan is
                # long done, so this never stalls ScalarE's exp stream
                if len(carry_jobs) >= 2:
                    pjb, pht = carry_jobs.pop(0)
                    newc = cpool.tile([128, N, 2], BF16, tag="carry")
                    nc.scalar.activation(newc[:, :, 0:1],
                                         pht[:, :, tcp - 1:tcp], AF.Identity)
                    nc.scalar.activation(newc[:, :, 1:2],
                                         pht[:, :, tcp - 1:tcp],
                                         AF.Copy, bias=0.0, scale=0.0)
                    carry_tiles[pjb] = newc
                hct = workpool.tile([128, N, tc], BF16, tag="work")
                nc.vector.tensor_mul(hct[:], ht[:, :, 2:], crep[:])
                # n-reduction tree on GPSIMD (otherwise idle); ping-pong
                # through bt, which is dead after the scan
                nc.gpsimd.tensor_add(bt[:, 0:8, 2:], hct[:, 0:8, :],
                                     hct[:, 8:16, :])
                nc.gpsimd.tensor_add(bt[:, 8:12, 2:], bt[:, 0:4, 2:],
                                     bt[:, 4:8, 2:])
                nc.gpsimd.tensor_add(bt[:, 12:14, 2:], bt[:, 8:10, 2:],
                                     bt[:, 10:12, 2:])
                yt = ypool.tile([128, tc], BF16, tag="y")
                nc.gpsimd.tensor_add(yt[:], bt[:, 12, 2:], bt[:, 13, 2:])
                # Dp*x on ACT (per-partition scale), final add on GPSIMD
                dpxt = ypool.tile([128, tc], BF16, tag="dpx")
                nc.scalar.activation(dpxt[:], xt_sb[jb][:, t0:t0 + tc],
                                     AF.Copy, scale=dp_sb[:, jb:jb + 1])
                ybt = ybfpool.tile([128, tc], BF16, tag="ybf")
                nc.gpsimd.tensor_add(ybt[:], dpxt[:], yt[:])
                y_tiles.append(ybt)

            while carry_jobs:
                pjb, pht = carry_jobs.pop(0)
                newc = cpool.tile([128, N, 2], BF16, tag="carry")
                nc.scalar.activation(newc[:, :, 0:1],
                                     pht[:, :, tcp - 1:tcp], AF.Identity)
                nc.scalar.activation(newc[:, :, 1:2],
                                     pht[:, :, tcp - 1:tcp],
                                     AF.Copy, bias=0.0, scale=0.0)
                carry_tiles[pjb] = newc

            # previous chunk's tail: y-finalization (DVE) + out-projection
            # (PE) + evac (ACT).  Deferred one chunk so none of it sits on
            # this chunk's critical path for any engine.
            if pending_out is not None:
                pybf, pt0 = pending_out
                for eb in range(NEB):
                    pso = psum.tile([128, tc], F32, tag="ps")
                    for kb in range(NJB):
                        nc.tensor.matmul(
                            pso[:],
                            wo_sb[kb][:, eb * 128:(eb + 1) * 128],
                            pybf[kb][:],
                            start=(kb == 0), stop=(kb == NJB - 1))
                    pot = popool.tile([128, tc], FP16, tag="po")
                    nc.scalar.activation(pot[:], pso[:], AF.Identity,
                                         bias=bo_sb[:, eb:eb + 1])
                    nc.sync.dma_start(
                        part[eb * 128:(eb + 1) * 128, pt0:pt0 + tc], pot[:])
            pending_out = (y_tiles, t0)

        pybf, pt0 = pending_out
        for eb in range(NEB):
            pso = psum.tile([128, tc], F32, tag="ps")
            for kb in range(NJB):
                nc.tensor.matmul(
                    pso[:],
                    wo_sb[kb][:, eb * 128:(eb + 1) * 128],
                    pybf[kb][:],
                    start=(kb == 0), stop=(kb == NJB - 1))
            pot = popool.tile([128, tc], FP16, tag="po")
            nc.scalar.activation(pot[:], pso[:], AF.Identity,
                                 bias=bo_sb[:, eb:eb + 1])
            nc.sync.dma_start(
                part[eb * 128:(eb + 1) * 128, pt0:pt0 + tc], pot[:])
    nc.compile()
    return nc


def _build_stage2(t_len=T):
    nc = bacc.Bacc("TRN2", target_bir_lowering=False, debug=False, num_devices=8)
    p0 = nc.dram_tensor("p0", [DL, t_len], FP16, kind="ExternalInput")
    p1 = nc.dram_tensor("p1", [DL, t_len], FP16, kind="ExternalInput")
    s = nc.dram_tensor("s", [DL, t_len], FP16, kind="ExternalOutput")
    tcw = 2048
    with tile.TileContext(nc) as tc_ctx, ExitStack() as ctx:
        pool = ctx.enter_context(tc_ctx.tile_pool(name="p", bufs=6))
        for kb in range(DL // 128):
            for i in range(t_len // tcw):
                t0 = i * tcw
                a_t = pool.tile([128, tcw], FP16, tag="a")
                nc.sync.dma_start(a_t[:], p0[kb * 128:(kb + 1) * 128, t0:t0 + tcw])
                b_t = pool.tile([128, tcw], FP16, tag="b")
                nc.sync.dma_start(b_t[:], p1[kb * 128:(kb + 1) * 128, t0:t0 + tcw])
                o_t = pool.tile([128, tcw], FP16, tag="o")
                nc.vector.tensor_add(o_t[:], a_t[:], b_t[:])
                nc.sync.dma_start(s[kb * 128:(kb + 1) * 128, t0:t0 + tcw], o_t[:])
    nc.compile()
    return nc


def _stage1_inputs(x, A_log, Dp, W_xproj, b_xproj, W_out, b_out):
    bf = ml_dtypes.bfloat16
    in_maps = []
    for c in range(8):
        b, j = c % 4, c // 4
        lo, hi = j * DL, (j + 1) * DL
        order = np.concatenate(
            [np.arange(lo, hi), np.arange(0, lo), np.arange(hi, D)])
        cols = np.concatenate([np.arange(lo, hi), np.arange(D, D + 2 * N)])
        xt_full = np.ascontiguousarray(x[b].T[order]).astype(bf)
        wxc = np.ascontiguousarray(W_xproj[order][:, cols]).astype(bf)
        bx_pad = np.zeros(5 * 128, np.float32)
        bx_pad[:PC] = b_xproj[cols]
        bx_arr = np.ascontiguousarray(bx_pad.reshape(5, 128).T)
        alog_l = np.ascontiguousarray(
            A_log[lo:hi].reshape(NJB, 128, N).transpose(1, 0, 2).reshape(128, NJB * N))
        dp_l = np.ascontiguousarray(Dp[lo:hi].reshape(NJB, 128).T)
        wo_l = np.ascontiguousarray(W_out[lo:hi]).astype(bf)
        bo_src = b_out if j == 0 else np.zeros_like(b_out)
        bo_l = np.ascontiguousarray(bo_src.reshape(NEB, 128).T.astype(np.float32))
        in_maps.append({
            "xt": xt_full, "wx": wxc, "bx": bx_arr, "alog": alog_l,
            "dp": dp_l, "wo": wo_l, "bo": bo_l,
            "ident": np.eye(128, dtype=bf),
        })
    return in_maps


def kernel(x, A_log, Dp, W_xproj, b_xproj, W_out, b_out, _trace=False):
    x = np.asarray(x, np.float32)
    A_log = np.asarray(A_log, np.float32)
    Dp = np.asarray(Dp, np.float32)
    W_xproj = np.asarray(W_xproj, np.float32)
    b_xproj = np.asarray(b_xproj, np.float32)
    W_out = np.asarray(W_out, np.float32)
    b_out = np.asarray(b_out, np.float32)

    if "s1" not in _cache:
        _cache["s1"] = _build_stage1()
    if "s2" not in _cache:
        _cache["s2"] = _build_stage2()

    in1 = _stage1_inputs(x, A_log, Dp, W_xproj, b_xproj, W_out, b_out)
    kw = dict(trace=True, trace_cores=list(range(8))) if _trace else {}
    res1 = run_bass_kernel_spmd(_cache["s1"], in1, core_ids=list(range(8)), **kw)
    parts = [res1.results[c]["part"] for c in range(8)]

    in2 = []
    for c in range(8):
        b, eh = c % 4, c // 4
        in2.append({
            "p0": np.ascontiguousarray(parts[b][eh * DL:(eh + 1) * DL]),
            "p1": np.ascontiguousarray(parts[4 + b][eh * DL:(eh + 1) * DL]),
        })
    res2 = run_bass_kernel_spmd(_cache["s2"], in2, core_ids=list(range(8)), **kw)

    outs = []
    for b in range(4):
        s0 = res2.results[b]["s"]
        s1 = res2.results[4 + b]["s"]
        outs.append(np.concatenate([s0, s1], axis=0).T)
    out = np.stack(outs).astype(np.float32)
    if _trace:
        return out, (res1, res2)
    return out



# revision 23
# speedup vs baseline: 1.0451x; 1.0451x over previous
"""Trainium2 Bass kernel for nn_MinimalSSM: selective-scan SSM block.

Reference computation (per batch b):
    proj  = x @ W_xproj + b_xproj                # [L, d+2n]
    delta = softplus(proj[:, :d])                # [L, d]
    Bm, Cm = proj[:, d:d+n], proj[:, d+n:]       # [L, n]
    A     = -exp(A_log)                          # [d, n]
    h_t   = exp(delta_t*A) * h_{t-1} + delta_t*Bm_t*x_t   (elementwise [d, n])
    y_t   = sum_n(h_t * Cm_t) + Dp * x_t
    out   = y @ W_out + b_out

Sharding (8 cores): batch (4) x d_model-half (2).  Each core computes the
full recurrence for its 512 channels of its batch, and a partial
out-projection (contraction over its d-half).  A second tiny kernel adds the
two partials per batch (sharded batch x out-column-half).

The time recurrence runs on VectorE's native tensor_tensor_scan
(state = a*state + b along the free dim, fp32 internal state).
exp(delta*A) is computed on ScalarE as activation(Exp, scale=A[:,n]) --
per-partition vector scale -- so the DVE only does the multiplies,
scan, and the n-reduction tree.

Layout inside a core: partition dim = 128-channel block (4 blocks),
free dims = (n=16, t=Tc) per time-chunk, t innermost/contiguous.
"""

import numpy as np
import ml_dtypes

import concourse.bacc as bacc
import concourse.bass as bass
import concourse.tile as tile
from concourse import mybir
from concourse.bass_utils import run_bass_kernel_spmd
from contextlib import ExitStack

F32 = mybir.dt.float32
BF16 = mybir.dt.bfloat16
FP16 = mybir.dt.float16
AF = mybir.ActivationFunctionType
OP = mybir.AluOpType

B, T, D, N = 4, 2048, 1024, 16
NP = N // 2          # interleaved n-segment pairs
DL = D // 2          # channels per core
NJB = DL // 128      # 4 local channel blocks
NKB = D // 128       # 8 contraction blocks for proj
NEB = D // 128       # 8 output-column blocks
PC = 544             # proj columns per core: 512 delta + 16 B + 16 C
TC = 256             # time chunk
NCH = T // TC

_cache = {}


def _register_scan2():
    """Register a hand-written custom-DVE op SSM_SCAN2_ANT:

        out[p, k] = state[k%2],  state[e] := in0[p,k]*state[e] + in1[p,k]

    i.e. TWO interleaved affine recurrences (even/odd stream elements).
    The stock TENSOR_TENSOR_SCAN routes the recurrence backward through
    the datapath (stage1 -> stage0) and stalls one bubble cycle per
    element (2 cyc/elem).  Interleaving two independent recurrences
    makes the natural 2-cycle feedback latency of the a-flop path
    (NEXT_ALU_OUT_A reads the next block's a-flop as of the previous
    cycle = state of element k-2) exactly right, so the op streams at
    1 elem/cycle -- 2x the stock scan.

    Layout trick: an AP [p, t, n2] over a [p, n2, t] tile (rearrange
    "p n t -> p t n") produces the interleaved stream of two adjacent
    n-segments, so callers keep the plain segment-major tiles.

    Seeding: a 2-cycle seed uOp writes 0 into the a-flop (without
    consuming the streams) so elements 0/1 read a finite value; the
    caller's in-band (a=0, b=carry) dummy columns then inject the real
    initial state.
    """
    import numpy as _np
    from concourse import dve_ops as _DOPS
    from concourse.dve_uop import (
        UopConfig, Trigger, InpSel, OutSel, OutPath, AluInp, DelayInp,
        ENABLE,
    )
    from concourse.dve_spec import AluOp as _AOp, Spec as _Spec, \
        Src0 as _S0, Src1 as _S1
    from concourse.dve_uop import DveOpSpec

    NAME = "SSM_SCAN2_ANT"
    if NAME in _DOPS._SUB_OPCODE_FOR_NAME:
        return next(o for o in _DOPS.OPS if o.name == NAME)

    def _ref(in0, in1, s0, s1, imm2):
        a = in0.astype(_np.float32)
        b = in1.astype(_np.float32)
        P = a.shape[0]
        a2, b2 = a.reshape(P, -1), b.reshape(P, -1)
        o2 = _np.zeros_like(b2)
        s = _np.zeros((P, 2), _np.float32)
        for k in range(a2.shape[1]):
            s[:, k % 2] = a2[:, k] * s[:, k % 2] + b2[:, k]
            o2[:, k] = s[:, k % 2]
        return o2.reshape(in0.shape)

    def _bypass_chain(u, first, last):
        for bi in range(first, last + 1):
            u.datapath_config[bi].pass_through_alu()

    # --- uop 0: seed.  2 cycles, no stream consumption; blk1 computes 0
    # (ZERO via blk0's delay chain 2) into its out+a flops.
    seed = UopConfig()
    seed.enable_input(InpSel.SRC_0, 1)
    seed.enable_input(InpSel.SRC_1, 2)
    seed.enable_input(InpSel.ZERO, 3)
    b0 = seed.datapath_config[0]
    b0.enable_delay_from_src(DelayInp.PREV_DELAY, 0)
    b0.enable_delay_from_src(DelayInp.PREV_DELAY, 1)
    b0.enable_delay_from_src(DelayInp.PREV_DELAY, 2)
    b0.pass_through_alu()
    b1 = seed.datapath_config[1]
    b1.enable_alu(_AOp.BYPASS, AluInp.PREV_DELAY_2, AluInp.PREV_DELAY_2)
    b1.alu_out_a_enable = ENABLE
    _bypass_chain(seed, 2, 7)
    seed.repeat_count = 2
    seed.trigger = (Trigger.COUNT, Trigger.NONE, Trigger.NONE)
    seed.next_uop = (1, 0, 0)

    # --- uop 1: steady.  1 elem/cycle:
    #   blk0: m_k = a_k * state_{k-2}   (NEXT_ALU_OUT_A = blk1 a-flop,
    #                                    written on the previous cycle)
    #   blk1: state_k = m_k + b_k       (-> out flop + a flop)
    st = UopConfig()
    st.enable_input(InpSel.SRC_0, 1)
    st.enable_input(InpSel.SRC_1, 2)
    s0b = st.datapath_config[0]
    s0b.enable_delay_from_src(DelayInp.PREV_DELAY, 0)
    s0b.enable_delay_from_src(DelayInp.PREV_DELAY, 1)
    s0b.enable_alu(_AOp.MULTIPLY, AluInp.PREV_DELAY_0, AluInp.NEXT_ALU_OUT_A)
    s1b = st.datapath_config[1]
    s1b.enable_alu(_AOp.ADD, AluInp.PREV_ALU_OUT, AluInp.PREV_DELAY_1)
    s1b.alu_out_a_enable = ENABLE
    _bypass_chain(st, 2, 7)
    st.require_inp0 = ENABLE
    st.require_inp1 = ENABLE
    st.enable_output(OutSel.ALU_OUT, OutPath.WR0_LO)
    st.trigger = (Trigger.SRC_TENSOR_DONE, Trigger.NONE, Trigger.NONE)
    st.next_uop = (0, 0, 0)

    row = _DOPS._CUSTOM_DVE_ROW_BASE + len(_DOPS.OPS)
    assert row < 0x20
    hand_spec = DveOpSpec(name=NAME, uops=[seed, st], opcode=row, rd1_en=True)
    hand_spec.validate("v3")

    class _HandOp:
        name = NAME
        subdim = False
        spec = _Spec(body=_S0 * _S1, reference=_ref)

        def compile(self, ver):
            assert ver == "v3", f"SSM_SCAN2_ANT only has a v3 program ({ver})"
            return hand_spec

    op = _HandOp()
    _DOPS.OPS.append(op)
    _DOPS._SUB_OPCODE_FOR_NAME[NAME] = row
    _DOPS.CUSTOM_DVE_SPECS[NAME] = op.spec
    return op


SCAN2 = _register_scan2()


def _pin_act_tables():
    """Restrict bacc's activation-table choices to the one set containing
    every function we use (Exp, Ln, Identity, MemsetZero) so the compiler
    never inserts mid-kernel ACT_TABLE_LOAD switches."""
    import concourse.bacc as _bacc_mod
    from concourse.hw_specs import get_activation_tables as _orig

    def _only_nl_exp(arch):
        tabs = _orig(arch)
        # keep every entry (act_func_set_id is positional) but empty out the
        # alternatives so the chooser can only pick the one full set
        return {k: (v if k == "natural_log_exp_and_others" else set())
                for k, v in tabs.items()}

    _bacc_mod.get_activation_tables = _only_nl_exp


_pin_act_tables()


def _build_stage1(t_len=T, tc=TC):
    nch = t_len // tc
    nc = bacc.Bacc("TRN2", target_bir_lowering=False, debug=False, num_devices=8)
    xt = nc.dram_tensor("xt", [D, t_len], BF16, kind="ExternalInput")
    wx = nc.dram_tensor("wx", [D, PC], BF16, kind="ExternalInput")
    bx = nc.dram_tensor("bx", [128, 5], F32, kind="ExternalInput")
    alog = nc.dram_tensor("alog", [128, NJB * N], F32, kind="ExternalInput")
    dp = nc.dram_tensor("dp", [128, NJB], F32, kind="ExternalInput")
    wo = nc.dram_tensor("wo", [DL, D], BF16, kind="ExternalInput")
    bo = nc.dram_tensor("bo", [128, NEB], F32, kind="ExternalInput")
    ident = nc.dram_tensor("ident", [128, 128], BF16, kind="ExternalInput")
    part = nc.dram_tensor("part", [D, t_len], FP16, kind="ExternalOutput")
    # [chunk, B/C, pair, t, parity] -- interleaved scan layout
    bc_dram = nc.dram_tensor("bc_scratch", [t_len // tc, 2, NP, tc, 2], BF16)

    with tile.TileContext(nc) as tc_ctx, ExitStack() as ctx:
        const = ctx.enter_context(tc_ctx.tile_pool(name="const", bufs=1))
        psum = ctx.enter_context(
            tc_ctx.tile_pool(name="psum", bufs=8, space="PSUM"))
        dpool = ctx.enter_context(tc_ctx.tile_pool(name="delta", bufs=6))
        dxpool = ctx.enter_context(tc_ctx.tile_pool(name="dx", bufs=4))
        bcpool = ctx.enter_context(tc_ctx.tile_pool(name="bc", bufs=2))
        reppool = ctx.enter_context(tc_ctx.tile_pool(name="rep", bufs=2))
        apool = ctx.enter_context(tc_ctx.tile_pool(name="apool", bufs=3))
        workpool = ctx.enter_context(tc_ctx.tile_pool(name="work", bufs=4))
        hpool = ctx.enter_context(tc_ctx.tile_pool(name="hpool", bufs=3))
        cpool = ctx.enter_context(tc_ctx.tile_pool(name="carry", bufs=8))
        ypool = ctx.enter_context(tc_ctx.tile_pool(name="y", bufs=4))
        ybfpool = ctx.enter_context(tc_ctx.tile_pool(name="ybf", bufs=8))
        popool = ctx.enter_context(tc_ctx.tile_pool(name="po", bufs=3))

        xt_sb = []
        for kb in range(NKB):
            tt = const.tile([128, t_len], BF16, tag=f"xt{kb}")
            nc.sync.dma_start(tt[:, 0:tc], xt[kb * 128:(kb + 1) * 128, 0:tc])
            xt_sb.append(tt)
        for kb in range(NKB):
            nc.sync.dma_start(xt_sb[kb][:, tc:],
                              xt[kb * 128:(kb + 1) * 128, tc:])
        wx_sb = []
        for kb in range(NKB):
            tt = const.tile([128, PC], BF16, tag=f"wx{kb}")
            nc.sync.dma_start(tt[:], wx[kb * 128:(kb + 1) * 128, :])
            wx_sb.append(tt)
        wo_sb = []
        for kb in range(NJB):
            tt = const.tile([128, D], BF16, tag=f"wo{kb}")
            nc.sync.dma_start(tt[:], wo[kb * 128:(kb + 1) * 128, :])
            wo_sb.append(tt)
        bx_sb = const.tile([128, 5], F32, tag="bx")
        nc.sync.dma_start(bx_sb[:], bx[:])
        bo_sb = const.tile([128, NEB], F32, tag="bo")
        nc.sync.dma_start(bo_sb[:], bo[:])
        dp_sb = const.tile([128, NJB], F32, tag="dp")
        nc.sync.dma_start(dp_sb[:], dp[:])
        alog_sb = const.tile([128, NJB * N], F32, tag="alog")
        nc.sync.dma_start(alog_sb[:], alog[:])
        aexp_sb = const.tile([128, NJB * N], F32, tag="aexp")
        nc.scalar.activation(aexp_sb[:], alog_sb[:], AF.Exp)
        aneg_sb = const.tile([128, NJB * N], F32, tag="aneg")
        nc.vector.tensor_scalar_mul(aneg_sb[:], aexp_sb[:], -1.0)
        id_sb = const.tile([128, 128], BF16, tag="ident")
        nc.sync.dma_start(id_sb[:], ident[:])
        # (0, 0), (1, 1) bf16 pattern for the interleaved-scan dummy cols
        const01 = const.tile([128, 2, 2], BF16, tag="const01")
        nc.vector.memset(const01[:, 0, :], 0.0)
        nc.vector.memset(const01[:, 1, :], 1.0)

        carry_tiles = [None] * NJB
        pending_out = None
        for ci in range(nch):
            t0 = ci * tc
            delta_tiles = []
            bct = None
            for m in (4, 0, 1, 2, 3):   # B/C first: unblocks the broadcasts
                mm = 128 if m < 4 else 32
                ps = psum.tile([mm, tc], F32, tag="ps")
                for kb in range(NKB):
                    nc.tensor.matmul(
                        ps[:],
                        wx_sb[kb][:, m * 128:m * 128 + mm],
                        xt_sb[kb][:, t0:t0 + tc],
                        start=(kb == 0), stop=(kb == NKB - 1))
                if m < 4:
                    # softplus(v) = ln(1 + exp(v)); Exp+Ln share one ACT table
                    et_ = dpool.tile([128, tc], F32, tag="etmp")
                    nc.scalar.activation(et_[:], ps[:], AF.Exp,
                                         bias=bx_sb[:, m:m + 1])
                    dt_ = dpool.tile([128, tc], BF16, tag="delta")
                    nc.scalar.activation(dt_[:], et_[:], AF.Ln, bias=1.0)
                    delta_tiles.append(dt_)
                else:
                    bct = bcpool.tile([32, tc], BF16, tag="bc")
                    nc.scalar.activation(bct[:], ps[:], AF.Identity,
                                         bias=bx_sb[:32, 4:5])
            # strided interleave writes: n = 8e + pr lands at [pr, t, e]
            for h in range(2):          # B rows then C rows
                for e in range(2):
                    nc.sync.dma_start(
                        bc_dram[ci, h, :, :, e],
                        bct[16 * h + NP * e:16 * h + NP * e + NP, :])
            # interleaved-pair layout [pair, t, e]: rep[p, pr, t, e] =
            # B[2*pr+e, t] -- one strided-row DMA per parity
            # n <-> (pair, parity) mapping: e = n // 8, pr = n % 8.
            # bc_dram holds the INTERLEAVED [pr, t, e] layout: the write
            # side pays the stride (8-partition DMAs -> only 2k small
            # descriptors each), so the 128-partition broadcast reads are
            # fully contiguous.
            brep = reppool.tile([128, NP, tc, 2], BF16, tag="brep")
            crep = reppool.tile([128, NP, tc, 2], BF16, tag="crep")
            nc.sync.dma_start(
                brep[:], bc_dram[ci, 0].partition_broadcast(128))
            nc.sync.dma_start(
                crep[:], bc_dram[ci, 1].partition_broadcast(128))

            y_tiles = []
            carry_jobs = []
            for jb in range(NJB):
                dt_ = delta_tiles[jb]
                # dx duplicated over the parity dim: dxi[t, e] = delta*x
                dxi = dxpool.tile([128, tc, 2], BF16, tag="dx")
                nc.vector.tensor_mul(
                    dxi[:],
                    dt_[:].unsqueeze(2).broadcast_to([128, tc, 2]),
                    xt_sb[jb][:, t0:t0 + tc].unsqueeze(2)
                    .broadcast_to([128, tc, 2]))
                # a/b tiles [128, pair, tcp, e] carry 2 leading dummy
                # t-columns per pair-segment: a = (0, 1), b = (carry, 0)
                # for BOTH parities.  One flat interleaved scan
                # (SSM_SCAN2_ANT: two accumulators, alternating elements)
                # chains all 8 pair-segments at 1 elem/cycle.
                tcp = tc + 2
                at = apool.tile([128, NP, tcp, 2], BF16, tag="a")
                nc.vector.tensor_copy(
                    at[:, :, 0:2, :].rearrange("p r t e -> p r (t e)"),
                    const01[:].rearrange("p t e -> p (t e)")
                    .unsqueeze(1).broadcast_to([128, NP, 4]))
                for n in range(N):
                    nc.scalar.activation(
                        at[:, n % NP, 2:, n // NP], dt_[:], AF.Exp,
                        scale=aneg_sb[:, jb * N + n:jb * N + n + 1])
                bt = workpool.tile([128, NP, tcp, 2], BF16, tag="work")
                if ci == 0:
                    nc.vector.memset(
                        bt[:, :, 0:2, :].rearrange("p r t e -> p r (t e)"),
                        0.0)
                else:
                    nc.vector.tensor_copy(
                        bt[:, :, 0:2, :].rearrange("p r t e -> p r (t e)"),
                        carry_tiles[jb][:].rearrange("p r t e -> p r (t e)"))
                dx_b = dxi[:].rearrange("p t e -> p (t e)") \
                    .unsqueeze(1).broadcast_to([128, NP, tc * 2])
                nc.vector.tensor_mul(
                    bt[:, :, 2:, :].rearrange("p r t e -> p r (t e)"),
                    dx_b, brep[:].rearrange("p r t e -> p r (t e)"))
                ht = hpool.tile([128, NP, tcp, 2], BF16, tag="h")
                nc.vector._custom_dve(
                    SCAN2,
                    out=ht[:].rearrange("p r t e -> p (r t e)"),
                    in0=at[:].rearrange("p r t e -> p (r t e)"),
                    in1=bt[:].rearrange("p r t e -> p (r t e)"))
                if ci < nch - 1:
                    carry_jobs.append((jb, ht))
                # extract the PREVIOUS jb's carry on ACT: by now its scan is
                # long done, so this never stalls ScalarE's exp stream
                if len(carry_jobs) >= 2:
                    pjb, pht = carry_jobs.pop(0)
                    newc = cpool.tile([128, NP, 2, 2], BF16, tag="carry")
                    nc.scalar.activation(newc[:, :, 0, :],
                                         pht[:, :, tcp - 1, :], AF.Identity)
                    nc.scalar.activation(newc[:, :, 1, :],
                                         pht[:, :, tcp - 1, :],
                                         AF.Copy, bias=0.0, scale=0.0)
                    carry_tiles[pjb] = newc
                hct = workpool.tile([128, NP, tc, 2], BF16, tag="work")
                nc.vector.tensor_mul(
                    hct[:].rearrange("p r t e -> p r (t e)"),
                    ht[:, :, 2:, :].rearrange("p r t e -> p r (t e)"),
                    crep[:].rearrange("p r t e -> p r (t e)"))
                # n-reduction tree over pairs (ping-pong through bt, dead
                # after the scan), then the final parity add
                bv = bt[:, :, 2:, :].rearrange("p r t e -> p r (t e)")
                hv = hct[:].rearrange("p r t e -> p r (t e)")
                nc.vector.tensor_add(bv[:, 0:4], hv[:, 0:4], hv[:, 4:8])
                nc.vector.tensor_add(bv[:, 4:6], bv[:, 0:2], bv[:, 2:4])
                nc.vector.tensor_add(bv[:, 6], bv[:, 4], bv[:, 5])
                yt = ypool.tile([128, tc], BF16, tag="y")
                nc.vector.tensor_add(yt[:], bt[:, 6, 2:, 0], bt[:, 6, 2:, 1])
                ybt = ybfpool.tile([128, tc], BF16, tag="ybf")
                nc.vector.scalar_tensor_tensor(
                    ybt[:], xt_sb[jb][:, t0:t0 + tc], dp_sb[:, jb:jb + 1],
                    yt[:], op0=OP.mult, op1=OP.add)
                y_tiles.append(ybt)

            while carry_jobs:
                pjb, pht = carry_jobs.pop(0)
                newc = cpool.tile([128, NP, 2, 2], BF16, tag="carry")
                nc.scalar.activation(newc[:, :, 0, :],
                                     pht[:, :, tcp - 1, :], AF.Identity)
                nc.scalar.activation(newc[:, :, 1, :],
                                     pht[:, :, tcp - 1, :],
                                     AF.Copy, bias=0.0, scale=0.0)
                carry_tiles[pjb] = newc

            # previous chunk's tail: y-finalization (DVE) + out-projection
            # (PE) + evac (ACT).  Deferred one chunk so none of it sits on
            # this chunk's critical path for any engine.
            if pending_out is not None:
                pybf, pt0 = pending_out
                for eb in range(NEB):
                    pso = psum.tile([128, tc], F32, tag="ps")
                    for kb in range(NJB):
                        nc.tensor.matmul(
                            pso[:],
                            wo_sb[kb][:, eb * 128:(eb + 1) * 128],
                            pybf[kb][:],
                            start=(kb == 0), stop=(kb == NJB - 1))
                    pot = popool.tile([128, tc], FP16, tag="po")
                    nc.scalar.activation(pot[:], pso[:], AF.Identity,
                                         bias=bo_sb[:, eb:eb + 1])
                    nc.sync.dma_start(
                        part[eb * 128:(eb + 1) * 128, pt0:pt0 + tc], pot[:])
            pending_out = (y_tiles, t0)

        pybf, pt0 = pending_out
        for eb in range(NEB):
            pso = psum.tile([128, tc], F32, tag="ps")
            for kb in range(NJB):
                nc.tensor.matmul(
                    pso[:],
                    wo_sb[kb][:, eb * 128:(eb + 1) * 128],
                    pybf[kb][:],
                    start=(kb == 0), stop=(kb == NJB - 1))
            pot = popool.tile([128, tc], FP16, tag="po")
            nc.scalar.activation(pot[:], pso[:], AF.Identity,
                                 bias=bo_sb[:, eb:eb + 1])
            nc.sync.dma_start(
                part[eb * 128:(eb + 1) * 128, pt0:pt0 + tc], pot[:])
    nc.compile()
    return nc


def _build_stage2(t_len=T):
    nc = bacc.Bacc("TRN2", target_bir_lowering=False, debug=False, num_devices=8)
    p0 = nc.dram_tensor("p0", [DL, t_len], FP16, kind="ExternalInput")
    p1 = nc.dram_tensor("p1", [DL, t_len], FP16, kind="ExternalInput")
    s = nc.dram_tensor("s", [DL, t_len], FP16, kind="ExternalOutput")
    tcw = 2048
    with tile.TileContext(nc) as tc_ctx, ExitStack() as ctx:
        pool = ctx.enter_context(tc_ctx.tile_pool(name="p", bufs=6))
        for kb in range(DL // 128):
            for i in range(t_len // tcw):
                t0 = i * tcw
                a_t = pool.tile([128, tcw], FP16, tag="a")
                nc.sync.dma_start(a_t[:], p0[kb * 128:(kb + 1) * 128, t0:t0 + tcw])
                b_t = pool.tile([128, tcw], FP16, tag="b")
                nc.sync.dma_start(b_t[:], p1[kb * 128:(kb + 1) * 128, t0:t0 + tcw])
                o_t = pool.tile([128, tcw], FP16, tag="o")
                nc.vector.tensor_add(o_t[:], a_t[:], b_t[:])
                nc.sync.dma_start(s[kb * 128:(kb + 1) * 128, t0:t0 + tcw], o_t[:])
    nc.compile()
    return nc


def _stage1_inputs(x, A_log, Dp, W_xproj, b_xproj, W_out, b_out):
    bf = ml_dtypes.bfloat16
    in_maps = []
    for c in range(8):
        b, j = c % 4, c // 4
        lo, hi = j * DL, (j + 1) * DL
        order = np.concatenate(
            [np.arange(lo, hi), np.arange(0, lo), np.arange(hi, D)])
        cols = np.concatenate([np.arange(lo, hi), np.arange(D, D + 2 * N)])
        xt_full = np.ascontiguousarray(x[b].T[order]).astype(bf)
        wxc = np.ascontiguousarray(W_xproj[order][:, cols]).astype(bf)
        bx_pad = np.zeros(5 * 128, np.float32)
        bx_pad[:PC] = b_xproj[cols]
        bx_arr = np.ascontiguousarray(bx_pad.reshape(5, 128).T)
        alog_l = np.ascontiguousarray(
            A_log[lo:hi].reshape(NJB, 128, N).transpose(1, 0, 2).reshape(128, NJB * N))
        dp_l = np.ascontiguousarray(Dp[lo:hi].reshape(NJB, 128).T)
        wo_l = np.ascontiguousarray(W_out[lo:hi]).astype(bf)
        bo_src = b_out if j == 0 else np.zeros_like(b_out)
        bo_l = np.ascontiguousarray(bo_src.reshape(NEB, 128).T.astype(np.float32))
        in_maps.append({
            "xt": xt_full, "wx": wxc, "bx": bx_arr, "alog": alog_l,
            "dp": dp_l, "wo": wo_l, "bo": bo_l,
            "ident": np.eye(128, dtype=bf),
        })
    return in_maps


def kernel(x, A_log, Dp, W_xproj, b_xproj, W_out, b_out, _trace=False):
    x = np.asarray(x, np.float32)
    A_log = np.asarray(A_log, np.float32)
    Dp = np.asarray(Dp, np.float32)
    W_xproj = np.asarray(W_xproj, np.float32)
    b_xproj = np.asarray(b_xproj, np.float32)
    W_out = np.asarray(W_out, np.float32)
    b_out = np.asarray(b_out, np.float32)

    if "s1" not in _cache:
        _cache["s1"] = _build_stage1()
    if "s2" not in _cache:
        _cache["s2"] = _build_stage2()

    in1 = _stage1_inputs(x, A_log, Dp, W_xproj, b_xproj, W_out, b_out)
    kw = dict(trace=True, trace_cores=list(range(8))) if _trace else {}
    res1 = run_bass_kernel_spmd(_cache["s1"], in1, core_ids=list(range(8)), **kw)
    parts = [res1.results[c]["part"] for c in range(8)]

    in2 = []
    for c in range(8):
        b, eh = c % 4, c // 4
        in2.append({
            "p0": np.ascontiguousarray(parts[b][eh * DL:(eh + 1) * DL]),
            "p1": np.ascontiguousarray(parts[4 + b][eh * DL:(eh + 1) * DL]),
        })
    res2 = run_bass_kernel_spmd(_cache["s2"], in2, core_ids=list(range(8)), **kw)

    outs = []
    for b in range(4):
        s0 = res2.results[b]["s"]
        s1 = res2.results[4 + b]["s"]
        outs.append(np.concatenate([s0, s1], axis=0).T)
    out = np.stack(outs).astype(np.float32)
    if _trace:
        return out, (res1, res2)
    return out



# revision 27
# speedup vs baseline: 1.5187x; 1.4531x over previous
"""Trainium2 Bass kernel for nn_MinimalSSM: selective-scan SSM block.

Reference computation (per batch b):
    proj  = x @ W_xproj + b_xproj                # [L, d+2n]
    delta = softplus(proj[:, :d])                # [L, d]
    Bm, Cm = proj[:, d:d+n], proj[:, d+n:]       # [L, n]
    A     = -exp(A_log)                          # [d, n]
    h_t   = exp(delta_t*A) * h_{t-1} + delta_t*Bm_t*x_t   (elementwise [d, n])
    y_t   = sum_n(h_t * Cm_t) + Dp * x_t
    out   = y @ W_out + b_out

Sharding (8 cores): batch (4) x d_model-half (2).  Each core computes the
full recurrence for its 512 channels of its batch, and a partial
out-projection (contraction over its d-half).  A second tiny kernel adds the
two partials per batch (sharded batch x out-column-half).

The time recurrence runs on VectorE's native tensor_tensor_scan
(state = a*state + b along the free dim, fp32 internal state).
exp(delta*A) is computed on ScalarE as activation(Exp, scale=A[:,n]) --
per-partition vector scale -- so the DVE only does the multiplies,
scan, and the n-reduction tree.

Layout inside a core: partition dim = 128-channel block (4 blocks),
free dims = (n=16, t=Tc) per time-chunk, t innermost/contiguous.
"""

import numpy as np
import ml_dtypes

import concourse.bacc as bacc
import concourse.bass as bass
import concourse.tile as tile
from concourse import mybir
from concourse.bass_utils import run_bass_kernel_spmd
from contextlib import ExitStack

F32 = mybir.dt.float32
BF16 = mybir.dt.bfloat16
FP16 = mybir.dt.float16
AF = mybir.ActivationFunctionType
OP = mybir.AluOpType

B, T, D, N = 4, 2048, 1024, 16
NP = N // 2          # interleaved n-segment pairs
DL = D // 2          # channels per core
NJB = DL // 128      # 4 local channel blocks
NKB = D // 128       # 8 contraction blocks for proj
NEB = D // 128       # 8 output-column blocks
PC = 544             # proj columns per core: 512 delta + 16 B + 16 C
TC = 256             # time chunk
NCH = T // TC

_cache = {}


def _register_scan2():
    """Register a hand-written custom-DVE op SSM_SCAN2_ANT:

        out[p, k] = state[k%2],  state[e] := in0[p,k]*state[e] + in1[p,k]

    i.e. TWO interleaved affine recurrences (even/odd stream elements).
    The stock TENSOR_TENSOR_SCAN routes the recurrence backward through
    the datapath (stage1 -> stage0) and stalls one bubble cycle per
    element (2 cyc/elem).  Interleaving two independent recurrences
    makes the natural 2-cycle feedback latency of the a-flop path
    (NEXT_ALU_OUT_A reads the next block's a-flop as of the previous
    cycle = state of element k-2) exactly right, so the op streams at
    1 elem/cycle -- 2x the stock scan.

    Layout trick: an AP [p, t, n2] over a [p, n2, t] tile (rearrange
    "p n t -> p t n") produces the interleaved stream of two adjacent
    n-segments, so callers keep the plain segment-major tiles.

    Seeding: a 2-cycle seed uOp writes 0 into the a-flop (without
    consuming the streams) so elements 0/1 read a finite value; the
    caller's in-band (a=0, b=carry) dummy columns then inject the real
    initial state.
    """
    import numpy as _np
    from concourse import dve_ops as _DOPS
    from concourse.dve_uop import (
        UopConfig, Trigger, InpSel, OutSel, OutPath, AluInp, DelayInp,
        ENABLE,
    )
    from concourse.dve_spec import AluOp as _AOp, Spec as _Spec, \
        Src0 as _S0, Src1 as _S1
    from concourse.dve_uop import DveOpSpec

    NAME = "SSM_SCAN2_ANT"
    if NAME in _DOPS._SUB_OPCODE_FOR_NAME:
        return next(o for o in _DOPS.OPS if o.name == NAME)

    def _ref(in0, in1, s0, s1, imm2):
        a = in0.astype(_np.float32)
        b = in1.astype(_np.float32)
        P = a.shape[0]
        a2, b2 = a.reshape(P, -1), b.reshape(P, -1)
        o2 = _np.zeros_like(b2)
        s = _np.zeros((P, 2), _np.float32)
        for k in range(a2.shape[1]):
            s[:, k % 2] = a2[:, k] * s[:, k % 2] + b2[:, k]
            o2[:, k] = s[:, k % 2]
        return o2.reshape(in0.shape)

    def _bypass_chain(u, first, last):
        for bi in range(first, last + 1):
            u.datapath_config[bi].pass_through_alu()

    # --- uop 0: seed.  2 cycles, no stream consumption; blk1 computes 0
    # (ZERO via blk0's delay chain 2) into its out+a flops.
    seed = UopConfig()
    seed.enable_input(InpSel.SRC_0, 1)
    seed.enable_input(InpSel.SRC_1, 2)
    seed.enable_input(InpSel.ZERO, 3)
    b0 = seed.datapath_config[0]
    b0.enable_delay_from_src(DelayInp.PREV_DELAY, 0)
    b0.enable_delay_from_src(DelayInp.PREV_DELAY, 1)
    b0.enable_delay_from_src(DelayInp.PREV_DELAY, 2)
    b0.pass_through_alu()
    b1 = seed.datapath_config[1]
    b1.enable_alu(_AOp.BYPASS, AluInp.PREV_DELAY_2, AluInp.PREV_DELAY_2)
    b1.alu_out_a_enable = ENABLE
    _bypass_chain(seed, 2, 7)
    seed.repeat_count = 2
    seed.trigger = (Trigger.COUNT, Trigger.NONE, Trigger.NONE)
    seed.next_uop = (1, 0, 0)

    # --- uop 1: steady.  1 elem/cycle:
    #   blk0: m_k = a_k * state_{k-2}   (NEXT_ALU_OUT_A = blk1 a-flop,
    #                                    written on the previous cycle)
    #   blk1: state_k = m_k + b_k       (-> out flop + a flop)
    st = UopConfig()
    st.enable_input(InpSel.SRC_0, 1)
    st.enable_input(InpSel.SRC_1, 2)
    s0b = st.datapath_config[0]
    s0b.enable_delay_from_src(DelayInp.PREV_DELAY, 0)
    s0b.enable_delay_from_src(DelayInp.PREV_DELAY, 1)
    s0b.enable_alu(_AOp.MULTIPLY, AluInp.PREV_DELAY_0, AluInp.NEXT_ALU_OUT_A)
    s1b = st.datapath_config[1]
    s1b.enable_alu(_AOp.ADD, AluInp.PREV_ALU_OUT, AluInp.PREV_DELAY_1)
    s1b.alu_out_a_enable = ENABLE
    _bypass_chain(st, 2, 7)
    st.require_inp0 = ENABLE
    st.require_inp1 = ENABLE
    st.enable_output(OutSel.ALU_OUT, OutPath.WR0_LO)
    st.trigger = (Trigger.SRC_TENSOR_DONE, Trigger.NONE, Trigger.NONE)
    st.next_uop = (0, 0, 0)

    row = _DOPS._CUSTOM_DVE_ROW_BASE + len(_DOPS.OPS)
    assert row < 0x20
    hand_spec = DveOpSpec(name=NAME, uops=[seed, st], opcode=row, rd1_en=True)
    hand_spec.validate("v3")

    class _HandOp:
        name = NAME
        subdim = False
        spec = _Spec(body=_S0 * _S1, reference=_ref)

        def compile(self, ver):
            assert ver == "v3", f"SSM_SCAN2_ANT only has a v3 program ({ver})"
            return hand_spec

    op = _HandOp()
    _DOPS.OPS.append(op)
    _DOPS._SUB_OPCODE_FOR_NAME[NAME] = row
    _DOPS.CUSTOM_DVE_SPECS[NAME] = op.spec
    return op


SCAN2 = _register_scan2()


def _pin_act_tables():
    """Restrict bacc's activation-table choices to the one set containing
    every function we use (Exp, Ln, Identity, MemsetZero) so the compiler
    never inserts mid-kernel ACT_TABLE_LOAD switches."""
    import concourse.bacc as _bacc_mod
    from concourse.hw_specs import get_activation_tables as _orig

    def _only_nl_exp(arch):
        tabs = _orig(arch)
        # keep every entry (act_func_set_id is positional) but empty out the
        # alternatives so the chooser can only pick the one full set
        return {k: (v if k == "natural_log_exp_and_others" else set())
                for k, v in tabs.items()}

    _bacc_mod.get_activation_tables = _only_nl_exp


_pin_act_tables()


def _build_stage1(t_len=T, tc=TC):
    nch = t_len // tc
    nc = bacc.Bacc("TRN2", target_bir_lowering=False, debug=False, num_devices=8)
    xt = nc.dram_tensor("xt", [D, t_len], BF16, kind="ExternalInput")
    wx = nc.dram_tensor("wx", [D, PC], BF16, kind="ExternalInput")
    bx = nc.dram_tensor("bx", [128, 5], F32, kind="ExternalInput")
    alog = nc.dram_tensor("alog", [128, NJB * N], F32, kind="ExternalInput")
    dp = nc.dram_tensor("dp", [128, NJB], F32, kind="ExternalInput")
    wo = nc.dram_tensor("wo", [DL, D], BF16, kind="ExternalInput")
    bo = nc.dram_tensor("bo", [128, NEB], F32, kind="ExternalInput")
    ident = nc.dram_tensor("ident", [128, 128], BF16, kind="ExternalInput")
    part = nc.dram_tensor("part", [D, t_len], FP16, kind="ExternalOutput")
    bc_dram = nc.dram_tensor("bc_scratch", [t_len // tc, 32, tc], BF16)

    with tile.TileContext(nc) as tc_ctx, ExitStack() as ctx:
        const = ctx.enter_context(tc_ctx.tile_pool(name="const", bufs=1))
        psum = ctx.enter_context(
            tc_ctx.tile_pool(name="psum", bufs=8, space="PSUM"))
        dpool = ctx.enter_context(tc_ctx.tile_pool(name="delta", bufs=6))
        dxpool = ctx.enter_context(tc_ctx.tile_pool(name="dx", bufs=4))
        bcpool = ctx.enter_context(tc_ctx.tile_pool(name="bc", bufs=2))
        reppool = ctx.enter_context(tc_ctx.tile_pool(name="rep", bufs=2))
        linpool = ctx.enter_context(tc_ctx.tile_pool(name="lin", bufs=2))
        apool = ctx.enter_context(tc_ctx.tile_pool(name="apool", bufs=3))
        workpool = ctx.enter_context(tc_ctx.tile_pool(name="work", bufs=4))
        hpool = ctx.enter_context(tc_ctx.tile_pool(name="hpool", bufs=3))
        cpool = ctx.enter_context(tc_ctx.tile_pool(name="carry", bufs=8))
        ypool = ctx.enter_context(tc_ctx.tile_pool(name="y", bufs=4))
        ybfpool = ctx.enter_context(tc_ctx.tile_pool(name="ybf", bufs=8))
        popool = ctx.enter_context(tc_ctx.tile_pool(name="po", bufs=3))

        xt_sb = []
        for kb in range(NKB):
            tt = const.tile([128, t_len], BF16, tag=f"xt{kb}")
            nc.sync.dma_start(tt[:, 0:tc], xt[kb * 128:(kb + 1) * 128, 0:tc])
            xt_sb.append(tt)
        for kb in range(NKB):
            nc.sync.dma_start(xt_sb[kb][:, tc:],
                              xt[kb * 128:(kb + 1) * 128, tc:])
        wx_sb = []
        for kb in range(NKB):
            tt = const.tile([128, PC], BF16, tag=f"wx{kb}")
            nc.sync.dma_start(tt[:], wx[kb * 128:(kb + 1) * 128, :])
            wx_sb.append(tt)
        wo_sb = []
        for kb in range(NJB):
            tt = const.tile([128, D], BF16, tag=f"wo{kb}")
            nc.sync.dma_start(tt[:], wo[kb * 128:(kb + 1) * 128, :])
            wo_sb.append(tt)
        bx_sb = const.tile([128, 5], F32, tag="bx")
        nc.sync.dma_start(bx_sb[:], bx[:])
        bo_sb = const.tile([128, NEB], F32, tag="bo")
        nc.sync.dma_start(bo_sb[:], bo[:])
        dp_sb = const.tile([128, NJB], F32, tag="dp")
        nc.sync.dma_start(dp_sb[:], dp[:])
        alog_sb = const.tile([128, NJB * N], F32, tag="alog")
        nc.sync.dma_start(alog_sb[:], alog[:])
        aexp_sb = const.tile([128, NJB * N], F32, tag="aexp")
        nc.scalar.activation(aexp_sb[:], alog_sb[:], AF.Exp)
        aneg_sb = const.tile([128, NJB * N], F32, tag="aneg")
        nc.vector.tensor_scalar_mul(aneg_sb[:], aexp_sb[:], -1.0)
        id_sb = const.tile([128, 128], BF16, tag="ident")
        nc.sync.dma_start(id_sb[:], ident[:])
        # (0, 0), (1, 1) bf16 pattern for the interleaved-scan dummy cols
        const01 = const.tile([128, 2, 2], BF16, tag="const01")
        nc.vector.memset(const01[:, 0, :], 0.0)
        nc.vector.memset(const01[:, 1, :], 1.0)

        carry_tiles = [None] * NJB
        pending_out = None
        for ci in range(nch):
            t0 = ci * tc
            delta_tiles = []
            bct = None
            for m in (4, 0, 1, 2, 3):   # B/C first: unblocks the broadcasts
                mm = 128 if m < 4 else 32
                ps = psum.tile([mm, tc], F32, tag="ps")
                for kb in range(NKB):
                    nc.tensor.matmul(
                        ps[:],
                        wx_sb[kb][:, m * 128:m * 128 + mm],
                        xt_sb[kb][:, t0:t0 + tc],
                        start=(kb == 0), stop=(kb == NKB - 1))
                if m < 4:
                    # softplus(v) = ln(1 + exp(v)); Exp+Ln share one ACT table
                    et_ = dpool.tile([128, tc], F32, tag="etmp")
                    nc.scalar.activation(et_[:], ps[:], AF.Exp,
                                         bias=bx_sb[:, m:m + 1])
                    dt_ = dpool.tile([128, tc], BF16, tag="delta")
                    nc.scalar.activation(dt_[:], et_[:], AF.Ln, bias=1.0)
                    delta_tiles.append(dt_)
                else:
                    bct = bcpool.tile([32, tc], BF16, tag="bc")
                    nc.scalar.activation(bct[:], ps[:], AF.Identity,
                                         bias=bx_sb[:32, 4:5])
            nc.sync.dma_start(bc_dram[ci], bct[:])
            # interleaved-pair layout [pair, t, e]: rep[p, pr, t, e] =
            # B[2*pr+e, t] -- one strided-row DMA per parity
            # n <-> (pair, parity) mapping: e = n // 8, pr = n % 8.
            # Broadcast-DMA the B/C rows linearly (big contiguous
            # descriptors), then build the interleaved [pair, t, e]
            # replicas with one strided Identity copy each on ScalarE --
            # strided DMA would shatter into 2-byte descriptors, and
            # GPSIMD/DVE copies are slower/contended.
            brep = reppool.tile([128, NP, tc, 2], BF16, tag="brep")
            crep = reppool.tile([128, NP, tc, 2], BF16, tag="crep")
            blin = linpool.tile([128, N, tc], BF16, tag="lin")
            nc.sync.dma_start(blin[:],
                              bc_dram[ci, 0:N, :].partition_broadcast(128))
            clin = linpool.tile([128, N, tc], BF16, tag="lin")
            nc.sync.dma_start(clin[:],
                              bc_dram[ci, N:2 * N, :].partition_broadcast(128))
            for rep, lin in ((brep, blin), (crep, clin)):
                nc.scalar.activation(
                    rep[:], lin[:].rearrange("p (e r) t -> p r t e", e=2),
                    AF.Identity)

            y_tiles = []
            carry_jobs = []
            for jb in range(NJB):
                dt_ = delta_tiles[jb]
                # dx duplicated over the parity dim: dxi[t, e] = delta*x
                dxi = dxpool.tile([128, tc, 2], BF16, tag="dx")
                nc.vector.tensor_mul(
                    dxi[:],
                    dt_[:].unsqueeze(2).broadcast_to([128, tc, 2]),
                    xt_sb[jb][:, t0:t0 + tc].unsqueeze(2)
                    .broadcast_to([128, tc, 2]))
                # a/b tiles [128, pair, tcp, e] carry 2 leading dummy
                # t-columns per pair-segment: a = (0, 1), b = (carry, 0)
                # for BOTH parities.  One flat interleaved scan
                # (SSM_SCAN2_ANT: two accumulators, alternating elements)
                # chains all 8 pair-segments at 1 elem/cycle.
                tcp = tc + 2
                at = apool.tile([128, NP, tcp, 2], BF16, tag="a")
                nc.vector.tensor_copy(
                    at[:, :, 0:2, :].rearrange("p r t e -> p r (t e)"),
                    const01[:].rearrange("p t e -> p (t e)")
                    .unsqueeze(1).broadcast_to([128, NP, 4]))
                for n in range(N):
                    nc.scalar.activation(
                        at[:, n % NP, 2:, n // NP], dt_[:], AF.Exp,
                        scale=aneg_sb[:, jb * N + n:jb * N + n + 1])
                bt = workpool.tile([128, NP, tcp, 2], BF16, tag="work")
                if ci == 0:
                    nc.vector.memset(
                        bt[:, :, 0:2, :].rearrange("p r t e -> p r (t e)"),
                        0.0)
                else:
                    nc.vector.tensor_copy(
                        bt[:, :, 0:2, :].rearrange("p r t e -> p r (t e)"),
                        carry_tiles[jb][:].rearrange("p r t e -> p r (t e)"))
                dx_b = dxi[:].rearrange("p t e -> p (t e)") \
                    .unsqueeze(1).broadcast_to([128, NP, tc * 2])
                nc.vector.tensor_mul(
                    bt[:, :, 2:, :].rearrange("p r t e -> p r (t e)"),
                    dx_b, brep[:].rearrange("p r t e -> p r (t e)"))
                ht = hpool.tile([128, NP, tcp, 2], BF16, tag="h")
                nc.vector._custom_dve(
                    SCAN2,
                    out=ht[:].rearrange("p r t e -> p (r t e)"),
                    in0=at[:].rearrange("p r t e -> p (r t e)"),
                    in1=bt[:].rearrange("p r t e -> p (r t e)"))
                if ci < nch - 1:
                    carry_jobs.append((jb, ht))
                # extract the PREVIOUS jb's carry on ACT: by now its scan is
                # long done, so this never stalls ScalarE's exp stream
                if len(carry_jobs) >= 2:
                    pjb, pht = carry_jobs.pop(0)
                    newc = cpool.tile([128, NP, 2, 2], BF16, tag="carry")
                    nc.scalar.activation(newc[:, :, 0, :],
                                         pht[:, :, tcp - 1, :], AF.Identity)
                    nc.scalar.activation(newc[:, :, 1, :],
                                         pht[:, :, tcp - 1, :],
                                         AF.Copy, bias=0.0, scale=0.0)
                    carry_tiles[pjb] = newc
                hct = workpool.tile([128, NP, tc, 2], BF16, tag="work")
                nc.vector.tensor_mul(
                    hct[:].rearrange("p r t e -> p r (t e)"),
                    ht[:, :, 2:, :].rearrange("p r t e -> p r (t e)"),
                    crep[:].rearrange("p r t e -> p r (t e)"))
                # n-reduction tree over pairs (ping-pong through bt, dead
                # after the scan), then the final parity add
                bv = bt[:, :, 2:, :].rearrange("p r t e -> p r (t e)")
                hv = hct[:].rearrange("p r t e -> p r (t e)")
                nc.vector.tensor_add(bv[:, 0:4], hv[:, 0:4], hv[:, 4:8])
                nc.vector.tensor_add(bv[:, 4:6], bv[:, 0:2], bv[:, 2:4])
                nc.vector.tensor_add(bv[:, 6], bv[:, 4], bv[:, 5])
                yt = ypool.tile([128, tc], BF16, tag="y")
                nc.vector.tensor_add(yt[:], bt[:, 6, 2:, 0], bt[:, 6, 2:, 1])
                ybt = ybfpool.tile([128, tc], BF16, tag="ybf")
                nc.vector.scalar_tensor_tensor(
                    ybt[:], xt_sb[jb][:, t0:t0 + tc], dp_sb[:, jb:jb + 1],
                    yt[:], op0=OP.mult, op1=OP.add)
                y_tiles.append(ybt)

            while carry_jobs:
                pjb, pht = carry_jobs.pop(0)
                newc = cpool.tile([128, NP, 2, 2], BF16, tag="carry")
                nc.scalar.activation(newc[:, :, 0, :],
                                     pht[:, :, tcp - 1, :], AF.Identity)
                nc.scalar.activation(newc[:, :, 1, :],
                                     pht[:, :, tcp - 1, :],
                                     AF.Copy, bias=0.0, scale=0.0)
                carry_tiles[pjb] = newc

            # previous chunk's tail: y-finalization (DVE) + out-projection
            # (PE) + evac (ACT).  Deferred one chunk so none of it sits on
            # this chunk's critical path for any engine.
            if pending_out is not None:
                pybf, pt0 = pending_out
                for eb in range(NEB):
                    pso = psum.tile([128, tc], F32, tag="ps")
                    for kb in range(NJB):
                        nc.tensor.matmul(
                            pso[:],
                            wo_sb[kb][:, eb * 128:(eb + 1) * 128],
                            pybf[kb][:],
                            start=(kb == 0), stop=(kb == NJB - 1))
                    pot = popool.tile([128, tc], FP16, tag="po")
                    nc.scalar.activation(pot[:], pso[:], AF.Identity,
                                         bias=bo_sb[:, eb:eb + 1])
                    nc.sync.dma_start(
                        part[eb * 128:(eb + 1) * 128, pt0:pt0 + tc], pot[:])
            pending_out = (y_tiles, t0)

        pybf, pt0 = pending_out
        for eb in range(NEB):
            pso = psum.tile([128, tc], F32, tag="ps")
            for kb in range(NJB):
                nc.tensor.matmul(
                    pso[:],
                    wo_sb[kb][:, eb * 128:(eb + 1) * 128],
                    pybf[kb][:],
                    start=(kb == 0), stop=(kb == NJB - 1))
            pot = popool.tile([128, tc], FP16, tag="po")
            nc.scalar.activation(pot[:], pso[:], AF.Identity,
                                 bias=bo_sb[:, eb:eb + 1])
            nc.sync.dma_start(
                part[eb * 128:(eb + 1) * 128, pt0:pt0 + tc], pot[:])
    nc.compile()
    return nc


def _build_stage2(t_len=T):
    nc = bacc.Bacc("TRN2", target_bir_lowering=False, debug=False, num_devices=8)
    p0 = nc.dram_tensor("p0", [DL, t_len], FP16, kind="ExternalInput")
    p1 = nc.dram_tensor("p1", [DL, t_len], FP16, kind="ExternalInput")
    s = nc.dram_tensor("s", [DL, t_len], FP16, kind="ExternalOutput")
    tcw = 2048
    with tile.TileContext(nc) as tc_ctx, ExitStack() as ctx:
        pool = ctx.enter_context(tc_ctx.tile_pool(name="p", bufs=6))
        for kb in range(DL // 128):
            for i in range(t_len // tcw):
                t0 = i * tcw
                a_t = pool.tile([128, tcw], FP16, tag="a")
                nc.sync.dma_start(a_t[:], p0[kb * 128:(kb + 1) * 128, t0:t0 + tcw])
                b_t = pool.tile([128, tcw], FP16, tag="b")
                nc.sync.dma_start(b_t[:], p1[kb * 128:(kb + 1) * 128, t0:t0 + tcw])
                o_t = pool.tile([128, tcw], FP16, tag="o")
                nc.vector.tensor_add(o_t[:], a_t[:], b_t[:])
                nc.sync.dma_start(s[kb * 128:(kb + 1) * 128, t0:t0 + tcw], o_t[:])
    nc.compile()
    return nc


def _stage1_inputs(x, A_log, Dp, W_xproj, b_xproj, W_out, b_out):
    bf = ml_dtypes.bfloat16
    in_maps = []
    for c in range(8):
        b, j = c % 4, c // 4
        lo, hi = j * DL, (j + 1) * DL
        order = np.concatenate(
            [np.arange(lo, hi), np.arange(0, lo), np.arange(hi, D)])
        cols = np.concatenate([np.arange(lo, hi), np.arange(D, D + 2 * N)])
        xt_full = np.ascontiguousarray(x[b].T[order]).astype(bf)
        wxc = np.ascontiguousarray(W_xproj[order][:, cols]).astype(bf)
        bx_pad = np.zeros(5 * 128, np.float32)
        bx_pad[:PC] = b_xproj[cols]
        bx_arr = np.ascontiguousarray(bx_pad.reshape(5, 128).T)
        alog_l = np.ascontiguousarray(
            A_log[lo:hi].reshape(NJB, 128, N).transpose(1, 0, 2).reshape(128, NJB * N))
        dp_l = np.ascontiguousarray(Dp[lo:hi].reshape(NJB, 128).T)
        wo_l = np.ascontiguousarray(W_out[lo:hi]).astype(bf)
        bo_src = b_out if j == 0 else np.zeros_like(b_out)
        bo_l = np.ascontiguousarray(bo_src.reshape(NEB, 128).T.astype(np.float32))
        in_maps.append({
            "xt": xt_full, "wx": wxc, "bx": bx_arr, "alog": alog_l,
            "dp": dp_l, "wo": wo_l, "bo": bo_l,
            "ident": np.eye(128, dtype=bf),
        })
    return in_maps


def kernel(x, A_log, Dp, W_xproj, b_xproj, W_out, b_out, _trace=False):
    x = np.asarray(x, np.float32)
    A_log = np.asarray(A_log, np.float32)
    Dp = np.asarray(Dp, np.float32)
    W_xproj = np.asarray(W_xproj, np.float32)
    b_xproj = np.asarray(b_xproj, np.float32)
    W_out = np.asarray(W_out, np.float32)
    b_out = np.asarray(b_out, np.float32)

    if "s1" not in _cache:
        _cache["s1"] = _build_stage1()
    if "s2" not in _cache:
        _cache["s2"] = _build_stage2()

    in1 = _stage1_inputs(x, A_log, Dp, W_xproj, b_xproj, W_out, b_out)
    kw = dict(trace=True, trace_cores=list(range(8))) if _trace else {}
    res1 = run_bass_kernel_spmd(_cache["s1"], in1, core_ids=list(range(8)), **kw)
    parts = [res1.results[c]["part"] for c in range(8)]

    in2 = []
    for c in range(8):
        b, eh = c % 4, c // 4
        in2.append({
            "p0": np.ascontiguousarray(parts[b][eh * DL:(eh + 1) * DL]),
            "p1": np.ascontiguousarray(parts[4 + b][eh * DL:(eh + 1) * DL]),
        })
    res2 = run_bass_kernel_spmd(_cache["s2"], in2, core_ids=list(range(8)), **kw)

    outs = []
    for b in range(4):
        s0 = res2.results[b]["s"]
        s1 = res2.results[4 + b]["s"]
        outs.append(np.concatenate([s0, s1], axis=0).T)
    out = np.stack(outs).astype(np.float32)
    if _trace:
        return out, (res1, res2)
    return out

